# revision 12
# baseline (speedup 1.0000x reference)
"""ChebyshevGCN Trainium2 kernel: 8-core row-parallel SpMM, transposed-space.

Math (per layer l in 0..1, poly order K=10):
    lap = -adj/deg[:,None]                     [N, N], N=8192
    Z_0 = X; Z_1 = lap@X; Z_k = 2*lap@Z_{k-1} - Z_{k-2}
    X = tanh(sum_k Z_k @ W[l,k] + b[l])

Distribution: core r owns rows r*1024..(r+1)*1024. The recurrence runs in
TRANSPOSED space: Zt_k = (lap@Z_{k-1})^T is produced as
    Zt[dblock, rows] += Zg[j, dblock]^T @ bp[j, rows]
with the resident lap^T column-block bp ([8192, 1024] bf16) as the MOVING
operand at free dim 512 — half the matmul instructions of the natural-space
form (256 vs 512 per step), amortizing per-instruction overhead. Zg (natural
layout, gathered from all cores each step) is the stationary operand. Local
Zt -> Zg natural transposes run on the DMA XBAR (dma_start transpose=True),
off the PE. The 1024-row block is split in two 512-row regions (one PSUM
bank each per dblock); region A's 64-j accumulation finishes mid-step so its
AllGather overlaps region B's sweep, and region B's tail is emitted after
the first stationary block of the next step's sweep. Y = sum_k Z_k W_k
accumulates in 4 pinned PSUM banks per layer straight from the bf16 Zt
tiles (lhsT), no extra transposes. bf16 inputs with fp32 PSUM accumulation
were validated bit-exact against the fp32 reference (tanh saturates).
"""

import os
import sys
from contextlib import ExitStack

for _p in ("/opt/trn_rl_repo", "/root/.axon_site/_ro/trn_rl_repo"):
    if os.path.isdir(_p) and _p not in sys.path:
        sys.path.append(_p)

import numpy as np
import ml_dtypes

from concourse import bacc, tile, bass_utils, mybir
from concourse.bass import _add_dep_helper

BF16 = ml_dtypes.bfloat16

N = 8192          # nodes
D = 256           # width
NCORES = 8
ROWS = N // NCORES          # 1024 local rows
P = 128                     # partitions
RH = ROWS // 2              # 512 region rows (one PSUM bank at f32)
HC = RH // P                # 4 local row chunks per region
JC = N // P                 # 64 contraction chunks
KPOLY = 10
NLAYERS = 2

_BUILT = None


def _build():
    nc = bacc.Bacc("TRN2", target_bir_lowering=False, debug=False,
                   num_devices=NCORES)
    f32 = mybir.dt.float32
    bf = mybir.dt.bfloat16

    bp_d = nc.dram_tensor("bp", [N, ROWS], bf, kind="ExternalInput").ap()
    # X pre-shuffled into the gathered layout used by the k=1 sweep:
    # xg[h][r*128+p, q*256+d] = X[r*1024 + h*512 + q*128 + p, d]
    xg_d = [nc.dram_tensor(f"xg{h}", [NCORES * P, HC * D], bf,
                           kind="ExternalInput").ap() for h in range(2)]
    xt_d = nc.dram_tensor("xt", [D, ROWS], bf, kind="ExternalInput").ap()
    w_d = nc.dram_tensor("w", [NLAYERS * KPOLY * 2, P, D], bf,
                         kind="ExternalInput").ap()
    b_d = nc.dram_tensor("b", [NLAYERS, ROWS, D], f32, kind="ExternalInput").ap()
    out_d = nc.dram_tensor("out", [ROWS, D], f32, kind="ExternalOutput").ap()

    rg = [list(range(NCORES))]
    COPY = mybir.ActivationFunctionType.Copy
    TANH = mybir.ActivationFunctionType.Tanh
    MUL = mybir.AluOpType.mult
    SUB = mybir.AluOpType.subtract
    ADD = mybir.AluOpType.add

    with tile.TileContext(nc) as tc, ExitStack() as ctx:
        bppool = ctx.enter_context(tc.tile_pool(name="bp", bufs=JC))
        cstpool = ctx.enter_context(tc.tile_pool(name="cst", bufs=1))
        ztpool = ctx.enter_context(tc.tile_pool(name="zt", bufs=4))
        zspool = ctx.enter_context(tc.tile_pool(name="zs", bufs=12))
        zgpool = ctx.enter_context(tc.tile_pool(name="zg", bufs=3))
        x1pool = ctx.enter_context(tc.tile_pool(name="x1", bufs=2))
        bpool = ctx.enter_context(tc.tile_pool(name="bb", bufs=1))
        tmppool = ctx.enter_context(tc.tile_pool(name="tmp", bufs=2))
        ocpool = ctx.enter_context(tc.tile_pool(name="oc", bufs=2))
        pspool = ctx.enter_context(tc.tile_pool(name="ps", bufs=2, space="PSUM"))
        ypool = ctx.enter_context(tc.tile_pool(name="y", bufs=1, space="PSUM"))
        dram = ctx.enter_context(tc.tile_pool(name="dram", bufs=8, space="DRAM"))

        # ---- lazily-issued residents (behind the first bp chunks so the
        # first sweep's matmuls start ASAP) ----
        cst = {}

        def get_cst():
            if not cst:
                w_sb = cstpool.tile([P, NLAYERS * KPOLY * 2, D], bf, name="w_sb")
                nc.scalar.dma_start(w_sb[:], w_d.rearrange("m p e -> p m e"))
                zt0 = ztpool.tile([P, 2, ROWS], bf, name="xt0", tag="zt")
                nc.scalar.dma_start(zt0[:], xt_d.rearrange("(dc p) i -> p dc i", p=P))
                cst["w"] = w_sb
                cst["zt0"] = zt0
            return cst

        # bp chunks DMA'd on first use so the 16MB resident load paces with
        # the first step's matmul sweep. Scalar HWDGE queue, so the zs loads
        # (sync queue) don't queue behind them.
        bp_src = bp_d.rearrange("(c p) i -> p c i", p=P)
        bp_sb = {}

        def get_bp(jc):
            if jc not in bp_sb:
                t = bppool.tile([P, ROWS], bf, name=f"bp{jc}", tag="bp")
                nc.scalar.dma_start(t[:], bp_src[:, jc, :])
                bp_sb[jc] = t
            return bp_sb[jc]

        def get_b(l, reg):
            t = bpool.tile([P, HC, D], f32, name=f"b{l}_{reg}", tag="b")
            nc.scalar.dma_start(
                t[:],
                b_d[l].rearrange("(c p) d -> p c d", p=P)
                [:, reg * HC:(reg + 1) * HC, :])
            return t

        def y_accum(Y, zt_t, l, k, ydeps, ics):
            # Y[:, ic, :] accumulates in pinned PSUM across the whole layer.
            # start clears has_written for a whole bank, so only the very
            # first matmul touching each bank (ic even, k==0, dc==0) sets it;
            # the odd-ic first matmul is ordered after it explicitly.
            w_sb = get_cst()["w"]
            for ic in ics:
                m = (l * KPOLY + k) * 2
                for dc in range(2):
                    mm = nc.tensor.matmul(
                        Y[:, ic, :], lhsT=zt_t[:, dc, ic * P:(ic + 1) * P],
                        rhs=w_sb[:, m + dc, :],
                        start=(k == 0 and dc == 0 and ic % 2 == 0),
                        stop=(k == KPOLY - 1 and dc == 1 and ic % 2 == 1),
                        skip_group_check=True)
                    if k == 0 and dc == 0:
                        if ic % 2 == 0:
                            ydeps[ic // 2] = mm
                        else:
                            _add_dep_helper(mm.ins, ydeps[ic // 2].ins, False,
                                            "bank-clear start runs first")

        def stt_region(zt_new, ps, zt_prev2, reg, k):
            # Zt_k[:, dc, region] = 2*ps[:, dc, :] - Zt_{k-2}[:, dc, region]
            # (k==1: plain copy, in1 ignored via bypass). All on DVE so the
            # scalar/sync DMA queues never gate the recurrence.
            sl = slice(reg * RH, (reg + 1) * RH)
            for dc in range(2):
                if k == 1:
                    # no Zt_{k-2} yet: plain DVE copy-with-cast
                    nc.vector.tensor_scalar_mul(
                        zt_new[:, dc, sl], ps[:, dc, :], 1.0)
                else:
                    nc.vector.scalar_tensor_tensor(
                        out=zt_new[:, dc, sl], in0=ps[:, dc, :],
                        scalar=2.0, in1=zt_prev2[:, dc, sl],
                        op0=MUL, op1=SUB)

        def xbar_natural(dst, src_zt, reg):
            # dst[p, c, dc*128:+128] = Zt[dc, reg*512 + c*128 + p] transposed
            # on the DMA XBAR, giving natural-layout [row, d] chunks. Sync
            # queue: only gather-launch DMAs live there, so a blocked head
            # never delays a gather.
            for c in range(HC):
                for dc in range(2):
                    nc.sync.dma_start(
                        dst[:, c, dc * P:(dc + 1) * P],
                        src_zt[:, dc, reg * RH + c * P: reg * RH + (c + 1) * P],
                        transpose=True)

        def gather(src, name, reg):
            # src: [P, HC, D] bf16 natural-layout region rows (zg or x1 tile)
            agi = dram.tile([P, HC * D], bf, name=f"agi_{name}",
                            tag=f"agi{reg}")
            nc.sync.dma_start(agi[:], src[:].rearrange("p c d -> p (c d)"))
            ago = dram.tile([NCORES * P, HC * D], bf, addr_space="Shared",
                            name=f"ago_{name}", tag=f"ago{reg}")
            nc.gpsimd.collective_compute(
                "AllGather", mybir.AluOpType.bypass, replica_groups=rg,
                ins=[agi[:].opt()], outs=[ago[:].opt()])
            return ago

        # xbar for X1 -> Zt_0' needs the natural->transposed direction:
        # zt0n[p(d), dc, reg*512 + c*128 + q(row)] = x1[q, c, dc*128 + p]
        def xbar_x1(zt0n, x1_t, reg):
            for c in range(HC):
                for dc in range(2):
                    nc.sync.dma_start(
                        zt0n[:, dc, reg * RH + c * P: reg * RH + (c + 1) * P],
                        x1_t[:, c, dc * P:(dc + 1) * P],
                        transpose=True)

        def finalize_region(l, Y, reg, b_t, zt0n):
            x1_t = None
            if l == 0:
                x1_t = x1pool.tile([P, HC, D], bf, name=f"x1_{reg}", tag="x1")
            for ci in range(HC):
                ic = reg * HC + ci
                tmp = tmppool.tile([P, D], f32, name=f"pre{l}_{ic}", tag="tmp")
                nc.vector.scalar_tensor_tensor(
                    out=tmp[:], in0=Y[:, ic, :], scalar=1.0,
                    in1=b_t[:, ci, :], op0=MUL, op1=ADD)
                if l == 0:
                    nc.scalar.activation(x1_t[:, ci, :], tmp[:], TANH)
                else:
                    oc = ocpool.tile([P, D], f32, name=f"oc{ic}", tag="oc")
                    nc.scalar.activation(oc[:], tmp[:], TANH)
                    nc.sync.dma_start(
                        out_d.rearrange("(c p) d -> p c d", p=P)[:, ic, :],
                        oc[:])
            if l == 0:
                xbar_x1(zt0n, x1_t, reg)
            return x1_t

        # pending region-B tail of the previous step, emitted after the
        # first stationary block of the next step's sweep so the PE has
        # work while STT-B / gather-B latency drains.
        pending = []

        def flush_pending():
            for fn in pending:
                fn()
            pending.clear()

        zt_prev1 = None  # Zt_{k-1} tile
        zt_prev2 = None  # Zt_{k-2} tile
        agout_prev = None  # [ago_reg0, ago_reg1] feeding the current sweep

        for l in range(NLAYERS):
            Y = ypool.tile([P, 2 * HC, D], f32, name=f"y{l}", tag="y")
            ydeps = {}

            for k in range(1, KPOLY):
                last = k == KPOLY - 1
                psA = pspool.tile([P, 2, RH], f32, name=f"ps{l}_{k}a", tag="ps")
                psB = pspool.tile([P, 2, RH], f32, name=f"ps{l}_{k}b", tag="ps")
                psR = [psA, psB]
                zs_sb = {}
                agout_next = [None, None]
                state = {}

                def get_zs(ph, r, l=l, k=k, zs_sb=zs_sb):
                    if (ph, r) not in zs_sb:
                        t = zspool.tile([P, HC * D], bf,
                                        name=f"zs{l}_{k}_{ph}_{r}", tag="zs")
                        if l == 0 and k == 1:
                            src = xg_d[ph][r * P:(r + 1) * P, :]
                        else:
                            src = agout_prev[ph][r * P:(r + 1) * P, :]
                        nc.scalar.dma_start(t[:], src)
                        zs_sb[(ph, r)] = t
                    return zs_sb[(ph, r)]

                # Region-major sweep: A(P1) A(P2) B(P1) B(P2). Region A's
                # accumulation closes at 50% of the step, so gather-A's
                # collective runs during region B's sweep and its output is
                # in SBUF well before the next step starts.
                nmm = 0
                for reg in range(2):
                    for ph in range(2):
                        for r in range(NCORES):
                            zst = get_zs(ph, r)
                            for q in range(HC):
                                jc = r * (2 * HC) + ph * HC + q
                                bp_t = get_bp(jc)
                                for dc in range(2):
                                    nc.tensor.matmul(
                                        psR[reg][:, dc, :],
                                        lhsT=zst[:, q * D + dc * P:
                                                 q * D + (dc + 1) * P],
                                        rhs=bp_t[:, reg * RH:(reg + 1) * RH],
                                        start=(ph == 0 and r == 0 and q == 0),
                                        stop=(ph == 1 and r == NCORES - 1
                                              and q == HC - 1),
                                        skip_group_check=True)
                            nmm += 1
                            if nmm == 1:
                                flush_pending()
                            if nmm == 4 and k == 1:
                                # k=0 ZW term: lhsT = Zt_0 (xt / xbar'd X1)
                                if zt_prev1 is None:
                                    zt_prev1 = get_cst()["zt0"]
                                zt_k = ztpool.tile([P, 2, ROWS], bf,
                                                   name=f"zt{l}_{k}", tag="zt")
                                state["zt_k"] = zt_k
                                y_accum(Y, zt_prev1, l, 0, ydeps,
                                        range(2 * HC))
                            elif nmm == 4 and k > 1:
                                zt_k = ztpool.tile([P, 2, ROWS], bf,
                                                   name=f"zt{l}_{k}", tag="zt")
                                state["zt_k"] = zt_k
                            if nmm == 8 and last:
                                state["b_t"] = get_b(l, 0)
                                state["b_t2"] = get_b(l, 1)
                                if l == 0:
                                    state["zt0n"] = ztpool.tile(
                                        [P, 2, ROWS], bf, name="zt0n", tag="zt")
                            if nmm == 17:
                                # one r-block of region B emitted: PE has
                                # cover for STT-A latency; now ZW-A (+ the
                                # layer finalize at k=9).
                                zt_k = state["zt_k"]
                                y_accum(Y, zt_k, l, k, ydeps, range(HC))
                                if last:
                                    x1A = finalize_region(
                                        l, Y, 0, state["b_t"],
                                        state.get("zt0n"))
                                    if l == 0:
                                        agout_next[0] = gather(
                                            x1A, f"x1_{l}_0", 0)
                    if reg == 0:
                        # region A accumulation complete: recurrence combine
                        # and (k<9) launch its all-gather mid-step.
                        zt_k = state["zt_k"]
                        stt_region(zt_k, psA, zt_prev2, 0, k)
                        if not last:
                            zgA = zgpool.tile([P, HC, D], bf,
                                              name=f"zg{l}_{k}0", tag="zg")
                            xbar_natural(zgA, zt_k, 0)
                            agout_next[0] = gather(zgA, f"{l}_{k}_0", 0)

                zt_k = state["zt_k"]

                def tail(l=l, k=k, last=last, zt_k=zt_k, psB=psB,
                         zt_prev2=zt_prev2, Y=Y, ydeps=ydeps,
                         agout_next=agout_next, state=state):
                    stt_region(zt_k, psB, zt_prev2, 1, k)
                    y_accum(Y, zt_k, l, k, ydeps, range(HC, 2 * HC))
                    if not last:
                        zgB = zgpool.tile([P, HC, D], bf,
                                          name=f"zg{l}_{k}1", tag="zg")
                        xbar_natural(zgB, zt_k, 1)
                        agout_next[1] = gather(zgB, f"{l}_{k}_1", 1)
                    else:
                        x1B = finalize_region(
                            l, Y, 1, state["b_t2"], state.get("zt0n"))
                        if l == 0:
                            agout_next[1] = gather(x1B, f"x1_{l}_1", 1)

                pending.append(tail)

                if last and l == 0:
                    zt_prev1, zt_prev2 = state["zt0n"], None
                else:
                    zt_prev2, zt_prev1 = zt_prev1, zt_k
                agout_prev = agout_next

        flush_pending()

    nc.compile()
    return nc


def _get_nc():
    global _BUILT
    if _BUILT is None:
        _BUILT = _build()
    return _BUILT


def kernel(X, adj_mat, degree, W, b):
    X = np.asarray(X, dtype=np.float32)
    adj_mat = np.asarray(adj_mat, dtype=np.float32)
    degree = np.asarray(degree, dtype=np.float32)
    W = np.asarray(W, dtype=np.float32)
    b = np.asarray(b, dtype=np.float32)

    nc = _get_nc()

    xbf = X.astype(BF16)
    # gathered layouts: xg[h][r*128+p, q*256+d] = X[r*1024 + h*512 + q*128 + p, d]
    x5 = xbf.reshape(NCORES, 2, HC, P, D)           # [r, h, q, p, d]
    xgs = [np.ascontiguousarray(
        x5[:, h].transpose(0, 2, 1, 3).reshape(NCORES * P, HC * D))
        for h in range(2)]
    wm = np.ascontiguousarray(
        W.reshape(NLAYERS * KPOLY, 2, P, D).reshape(NLAYERS * KPOLY * 2, P, D)
    ).astype(BF16)

    in_maps = []
    for r in range(NCORES):
        rows = slice(r * ROWS, (r + 1) * ROWS)
        lap_blk = (-adj_mat[rows] / degree[rows, None]).astype(BF16)   # [ROWS, N]
        bp = np.ascontiguousarray(lap_blk.T)                           # [N, ROWS]
        xloc = xbf[rows]
        in_maps.append({
            "bp": bp,
            "xg0": xgs[0],
            "xg1": xgs[1],
            "xt": np.ascontiguousarray(xloc.T),
            "w": wm,
            "b": np.ascontiguousarray(b[:, rows, :]),
        })

    res = bass_utils.run_bass_kernel_spmd(
        nc, in_maps, core_ids=list(range(NCORES)),
        trace=bool(int(os.environ.get("CHEB_TRACE", "0"))))
    kernel.last_exec_time_ns = res.exec_time_ns
    out = np.concatenate([res.results[r]["out"] for r in range(NCORES)], axis=0)
    return out


kernel.last_exec_time_ns = None


# revision 18
# speedup vs baseline: 1.0951x; 1.0951x over previous
"""ChebyshevGCN Trainium2 kernel: 8-core row-parallel SpMM, transposed-space.

Math (per layer l in 0..1, poly order K=10):
    lap = -adj/deg[:,None]                     [N, N], N=8192
    Z_0 = X; Z_1 = lap@X; Z_k = 2*lap@Z_{k-1} - Z_{k-2}
    X = tanh(sum_k Z_k @ W[l,k] + b[l])

Distribution: core r owns rows r*1024..(r+1)*1024. The recurrence runs in
TRANSPOSED space: Zt_k = (lap@Z_{k-1})^T is produced as
    Zt[dblock, rows] += Zg[j, dblock]^T @ bp[j, rows]
with the resident lap^T column-block bp ([8192, 1024] bf16) as the MOVING
operand at free dim 512 — half the matmul instructions of the natural-space
form (256 vs 512 per step), amortizing per-instruction overhead. Zg (natural
layout, gathered from all cores each step) is the stationary operand. Local
Zt -> Zg natural transposes run on the DMA XBAR (dma_start transpose=True),
off the PE. The 1024-row block is split in two 512-row regions (one PSUM
bank each per dblock); region A's 64-j accumulation finishes mid-step so its
AllGather overlaps region B's sweep, and region B's tail is emitted after
the first stationary block of the next step's sweep. Y = sum_k Z_k W_k
accumulates in 4 pinned PSUM banks per layer straight from the bf16 Zt
tiles (lhsT), no extra transposes. bf16 inputs with fp32 PSUM accumulation
were validated bit-exact against the fp32 reference (tanh saturates).
"""

import os
import sys
from contextlib import ExitStack

for _p in ("/opt/trn_rl_repo", "/root/.axon_site/_ro/trn_rl_repo"):
    if os.path.isdir(_p) and _p not in sys.path:
        sys.path.append(_p)

import numpy as np
import ml_dtypes

from concourse import bacc, tile, bass_utils, mybir
from concourse.bass import _add_dep_helper

BF16 = ml_dtypes.bfloat16

N = 8192          # nodes
D = 256           # width
NCORES = 8
ROWS = N // NCORES          # 1024 local rows
P = 128                     # partitions
RH = ROWS // 2              # 512 region rows (one PSUM bank at f32)
HC = RH // P                # 4 local row chunks per region
JC = N // P                 # 64 contraction chunks
KPOLY = 10
NLAYERS = 2

_BUILT = None


def _build():
    nc = bacc.Bacc("TRN2", target_bir_lowering=False, debug=False,
                   num_devices=NCORES)
    f32 = mybir.dt.float32
    bf = mybir.dt.bfloat16

    bp_d = nc.dram_tensor("bp", [N, ROWS], bf, kind="ExternalInput").ap()
    # X pre-shuffled into the gathered layout used by the k=1 sweep:
    # xg[h][r*128+p, q*256+d] = X[r*1024 + h*512 + q*128 + p, d]
    xg_d = [nc.dram_tensor(f"xg{h}", [NCORES * P, HC * D], bf,
                           kind="ExternalInput").ap() for h in range(2)]
    xt_d = nc.dram_tensor("xt", [D, ROWS], bf, kind="ExternalInput").ap()
    w_d = nc.dram_tensor("w", [NLAYERS * KPOLY * 2, P, D], bf,
                         kind="ExternalInput").ap()
    b_d = nc.dram_tensor("b", [NLAYERS, ROWS, D], f32, kind="ExternalInput").ap()
    out_d = nc.dram_tensor("out", [ROWS, D], f32, kind="ExternalOutput").ap()

    rg = [list(range(NCORES))]
    COPY = mybir.ActivationFunctionType.Copy
    TANH = mybir.ActivationFunctionType.Tanh
    MUL = mybir.AluOpType.mult
    SUB = mybir.AluOpType.subtract
    ADD = mybir.AluOpType.add

    with tile.TileContext(nc) as tc, ExitStack() as ctx:
        bppool = ctx.enter_context(tc.tile_pool(name="bp", bufs=JC))
        cstpool = ctx.enter_context(tc.tile_pool(name="cst", bufs=1))
        wpool = ctx.enter_context(tc.tile_pool(name="w", bufs=3))
        ztpool = ctx.enter_context(tc.tile_pool(name="zt", bufs=4))
        zspool = ctx.enter_context(tc.tile_pool(name="zs", bufs=16))
        zgpool = ctx.enter_context(tc.tile_pool(name="zg", bufs=3))
        x1pool = ctx.enter_context(tc.tile_pool(name="x1", bufs=2))
        bpool = ctx.enter_context(tc.tile_pool(name="bb", bufs=1))
        tmppool = ctx.enter_context(tc.tile_pool(name="tmp", bufs=2))
        ocpool = ctx.enter_context(tc.tile_pool(name="oc", bufs=2))
        pspool = ctx.enter_context(tc.tile_pool(name="ps", bufs=2, space="PSUM"))
        ypool = ctx.enter_context(tc.tile_pool(name="y", bufs=1, space="PSUM"))
        dram = ctx.enter_context(tc.tile_pool(name="dram", bufs=8, space="DRAM"))

        # ---- lazily-issued residents (behind the first bp chunks so the
        # first sweep's matmuls start ASAP) ----
        cst = {}

        def get_cst():
            if not cst:
                zt0 = ztpool.tile([P, 2, ROWS], bf, name="xt0", tag="zt")
                nc.scalar.dma_start(zt0[:], xt_d.rearrange("(dc p) i -> p dc i", p=P))
                cst["zt0"] = zt0
            return cst

        # W streamed per (layer, k): a [P, 2, D] tile each, prefetched one
        # step ahead. Frees ~2.2MiB of SBUF vs keeping all 40 resident.
        w_sb = {}
        w_src = w_d.rearrange("(m dc) p e -> p m dc e", dc=2)

        def get_w(l, k):
            if (l, k) not in w_sb:
                t = wpool.tile([P, 2, D], bf, name=f"w{l}_{k}", tag="w")
                nc.scalar.dma_start(t[:], w_src[:, l * KPOLY + k, :, :])
                w_sb[(l, k)] = t
            return w_sb[(l, k)]

        # bp chunks DMA'd on first use so the 16MB resident load paces with
        # the first step's matmul sweep. Scalar HWDGE queue, so the zs loads
        # (sync queue) don't queue behind them.
        bp_src = bp_d.rearrange("(c p) i -> p c i", p=P)
        bp_sb = {}

        def get_bp(jc):
            if jc not in bp_sb:
                t = bppool.tile([P, ROWS], bf, name=f"bp{jc}", tag="bp")
                nc.scalar.dma_start(t[:], bp_src[:, jc, :])
                bp_sb[jc] = t
            return bp_sb[jc]

        def get_b(l, reg):
            t = bpool.tile([P, HC, D], f32, name=f"b{l}_{reg}", tag="b")
            nc.scalar.dma_start(
                t[:],
                b_d[l].rearrange("(c p) d -> p c d", p=P)
                [:, reg * HC:(reg + 1) * HC, :])
            return t

        def y_accum(Y, zt_t, l, k, ydeps, ics):
            # Y[:, ic, :] accumulates in pinned PSUM across the whole layer.
            # start clears has_written for a whole bank, so only the very
            # first matmul touching each bank (ic even, k==0, dc==0) sets it;
            # the odd-ic first matmul is ordered after it explicitly.
            w_t = get_w(l, k)
            for ic in ics:
                for dc in range(2):
                    mm = nc.tensor.matmul(
                        Y[:, ic, :], lhsT=zt_t[:, dc, ic * P:(ic + 1) * P],
                        rhs=w_t[:, dc, :],
                        start=(k == 0 and dc == 0 and ic % 2 == 0),
                        stop=(k == KPOLY - 1 and dc == 1 and ic % 2 == 1),
                        skip_group_check=True)
                    if k == 0 and dc == 0:
                        if ic % 2 == 0:
                            ydeps[ic // 2] = mm
                        else:
                            _add_dep_helper(mm.ins, ydeps[ic // 2].ins, False,
                                            "bank-clear start runs first")

        def stt_region(zt_new, ps, zt_prev2, reg, k):
            # Zt_k[:, dc, region] = 2*ps[:, dc, :] - Zt_{k-2}[:, dc, region]
            # (k==1: plain copy, in1 ignored via bypass). All on DVE so the
            # scalar/sync DMA queues never gate the recurrence.
            sl = slice(reg * RH, (reg + 1) * RH)
            for dc in range(2):
                if k == 1:
                    # no Zt_{k-2} yet: plain DVE copy-with-cast
                    nc.vector.tensor_scalar_mul(
                        zt_new[:, dc, sl], ps[:, dc, :], 1.0)
                else:
                    nc.vector.scalar_tensor_tensor(
                        out=zt_new[:, dc, sl], in0=ps[:, dc, :],
                        scalar=2.0, in1=zt_prev2[:, dc, sl],
                        op0=MUL, op1=SUB)

        def xbar_natural(dst, src_zt, reg):
            # dst[p, c, dc*128:+128] = Zt[dc, reg*512 + c*128 + p] transposed
            # on the DMA XBAR (one instruction per dblock: the 3D output AP
            # folds transposed rows as r = c*128 + p). Sync queue: only
            # gather-launch DMAs live there, so a blocked head never delays
            # a gather.
            for dc in range(2):
                nc.sync.dma_start(
                    dst[:, :, dc * P:(dc + 1) * P],
                    src_zt[:, dc, reg * RH:(reg + 1) * RH],
                    transpose=True)

        def gather(src, name, reg):
            # src: [P, HC, D] bf16 natural-layout region rows (zg or x1 tile)
            agi = dram.tile([P, HC * D], bf, name=f"agi_{name}",
                            tag=f"agi{reg}")
            nc.sync.dma_start(agi[:], src[:].rearrange("p c d -> p (c d)"))
            ago = dram.tile([NCORES * P, HC * D], bf, addr_space="Shared",
                            name=f"ago_{name}", tag=f"ago{reg}")
            nc.gpsimd.collective_compute(
                "AllGather", mybir.AluOpType.bypass, replica_groups=rg,
                ins=[agi[:].opt()], outs=[ago[:].opt()])
            return ago

        # xbar for X1 -> Zt_0' needs the natural->transposed direction:
        # zt0n[p(d), dc, reg*512 + c*128 + q(row)] = x1[q, c, dc*128 + p]
        def xbar_x1(zt0n, x1_t, reg):
            # natural X1 chunk [128 rows, 256 d] -> zt0n[d-part, dc, rows]
            # (3D out AP: out[p, dc, k] = in[k, dc*128+p])
            for c in range(HC):
                nc.sync.dma_start(
                    zt0n[:, :, reg * RH + c * P: reg * RH + (c + 1) * P],
                    x1_t[:, c, :],
                    transpose=True)

        def finalize_region(l, Y, reg, b_t, zt0n):
            x1_t = None
            if l == 0:
                x1_t = x1pool.tile([P, HC, D], bf, name=f"x1_{reg}", tag="x1")
            for ci in range(HC):
                ic = reg * HC + ci
                tmp = tmppool.tile([P, D], f32, name=f"pre{l}_{ic}", tag="tmp")
                nc.vector.scalar_tensor_tensor(
                    out=tmp[:], in0=Y[:, ic, :], scalar=1.0,
                    in1=b_t[:, ci, :], op0=MUL, op1=ADD)
                if l == 0:
                    nc.scalar.activation(x1_t[:, ci, :], tmp[:], TANH)
                else:
                    oc = ocpool.tile([P, D], f32, name=f"oc{ic}", tag="oc")
                    nc.scalar.activation(oc[:], tmp[:], TANH)
                    nc.sync.dma_start(
                        out_d.rearrange("(c p) d -> p c d", p=P)[:, ic, :],
                        oc[:])
            if l == 0:
                xbar_x1(zt0n, x1_t, reg)
            return x1_t

        # pending region-B tail of the previous step, emitted after the
        # first stationary block of the next step's sweep so the PE has
        # work while STT-B / gather-B latency drains.
        pending = []

        def flush_pending():
            for fn in pending:
                fn()
            pending.clear()

        zt_prev1 = None  # Zt_{k-1} tile
        zt_prev2 = None  # Zt_{k-2} tile
        agout_prev = None  # [ago_reg0, ago_reg1] feeding the current sweep

        for l in range(NLAYERS):
            Y = ypool.tile([P, 2 * HC, D], f32, name=f"y{l}", tag="y")
            ydeps = {}

            for k in range(1, KPOLY):
                last = k == KPOLY - 1
                psA = pspool.tile([P, 2, RH], f32, name=f"ps{l}_{k}a", tag="ps")
                psB = pspool.tile([P, 2, RH], f32, name=f"ps{l}_{k}b", tag="ps")
                psR = [psA, psB]
                zs_sb = {}
                agout_next = [None, None]
                state = {}

                def get_zs(ph, r, l=l, k=k, zs_sb=zs_sb):
                    if (ph, r) not in zs_sb:
                        t = zspool.tile([P, HC * D], bf,
                                        name=f"zs{l}_{k}_{ph}_{r}", tag="zs")
                        if l == 0 and k == 1:
                            src = xg_d[ph][r * P:(r + 1) * P, :]
                        else:
                            src = agout_prev[ph][r * P:(r + 1) * P, :]
                        nc.scalar.dma_start(t[:], src)
                        zs_sb[(ph, r)] = t
                    return zs_sb[(ph, r)]

                # Region-major sweep: A(P1) A(P2) B(P1) B(P2). Region A's
                # accumulation closes at 50% of the step, so gather-A's
                # collective runs during region B's sweep and its output is
                # in SBUF well before the next step starts.
                nmm = 0
                for reg in range(2):
                    for ph in range(2):
                        for r in range(NCORES):
                            zst = get_zs(ph, r)
                            for q in range(HC):
                                jc = r * (2 * HC) + ph * HC + q
                                bp_t = get_bp(jc)
                                for dc in range(2):
                                    nc.tensor.matmul(
                                        psR[reg][:, dc, :],
                                        lhsT=zst[:, q * D + dc * P:
                                                 q * D + (dc + 1) * P],
                                        rhs=bp_t[:, reg * RH:(reg + 1) * RH],
                                        start=(ph == 0 and r == 0 and q == 0),
                                        stop=(ph == 1 and r == NCORES - 1
                                              and q == HC - 1),
                                        skip_group_check=True)
                            nmm += 1
                            if nmm == 1:
                                flush_pending()
                            if nmm == 4 and k == 1:
                                # k=0 ZW term: lhsT = Zt_0 (xt / xbar'd X1)
                                if zt_prev1 is None:
                                    zt_prev1 = get_cst()["zt0"]
                                zt_k = ztpool.tile([P, 2, ROWS], bf,
                                                   name=f"zt{l}_{k}", tag="zt")
                                state["zt_k"] = zt_k
                                y_accum(Y, zt_prev1, l, 0, ydeps,
                                        range(2 * HC))
                            elif nmm == 4 and k > 1:
                                zt_k = ztpool.tile([P, 2, ROWS], bf,
                                                   name=f"zt{l}_{k}", tag="zt")
                                state["zt_k"] = zt_k
                            if nmm == 8:
                                get_w(l, k)  # prefetch this step's ZW weights
                                if last:
                                    state["b_t"] = get_b(l, 0)
                                    state["b_t2"] = get_b(l, 1)
                                    if l == 0:
                                        state["zt0n"] = ztpool.tile(
                                            [P, 2, ROWS], bf, name="zt0n",
                                            tag="zt")
                            if nmm == 12 and last and l == 0:
                                get_w(1, 0)  # next layer's k=0 ZW weights
                            if nmm == 17:
                                # one r-block of region B emitted: PE has
                                # cover for STT-A latency; now ZW-A (+ the
                                # layer finalize at k=9).
                                zt_k = state["zt_k"]
                                y_accum(Y, zt_k, l, k, ydeps, range(HC))
                                if last:
                                    x1A = finalize_region(
                                        l, Y, 0, state["b_t"],
                                        state.get("zt0n"))
                                    if l == 0:
                                        agout_next[0] = gather(
                                            x1A, f"x1_{l}_0", 0)
                    if reg == 0:
                        # region A accumulation complete: recurrence combine
                        # and (k<9) launch its all-gather mid-step.
                        zt_k = state["zt_k"]
                        stt_region(zt_k, psA, zt_prev2, 0, k)
                        if not last:
                            zgA = zgpool.tile([P, HC, D], bf,
                                              name=f"zg{l}_{k}0", tag="zg")
                            xbar_natural(zgA, zt_k, 0)
                            agout_next[0] = gather(zgA, f"{l}_{k}_0", 0)

                zt_k = state["zt_k"]

                def tail(l=l, k=k, last=last, zt_k=zt_k, psB=psB,
                         zt_prev2=zt_prev2, Y=Y, ydeps=ydeps,
                         agout_next=agout_next, state=state):
                    stt_region(zt_k, psB, zt_prev2, 1, k)
                    y_accum(Y, zt_k, l, k, ydeps, range(HC, 2 * HC))
                    if not last:
                        zgB = zgpool.tile([P, HC, D], bf,
                                          name=f"zg{l}_{k}1", tag="zg")
                        xbar_natural(zgB, zt_k, 1)
                        agout_next[1] = gather(zgB, f"{l}_{k}_1", 1)
                    else:
                        x1B = finalize_region(
                            l, Y, 1, state["b_t2"], state.get("zt0n"))
                        if l == 0:
                            agout_next[1] = gather(x1B, f"x1_{l}_1", 1)

                pending.append(tail)

                if last and l == 0:
                    zt_prev1, zt_prev2 = state["zt0n"], None
                else:
                    zt_prev2, zt_prev1 = zt_prev1, zt_k
                agout_prev = agout_next

        flush_pending()

    nc.compile()
    return nc


def _get_nc():
    global _BUILT
    if _BUILT is None:
        _BUILT = _build()
    return _BUILT


def kernel(X, adj_mat, degree, W, b):
    X = np.asarray(X, dtype=np.float32)
    adj_mat = np.asarray(adj_mat, dtype=np.float32)
    degree = np.asarray(degree, dtype=np.float32)
    W = np.asarray(W, dtype=np.float32)
    b = np.asarray(b, dtype=np.float32)

    nc = _get_nc()

    xbf = X.astype(BF16)
    # gathered layouts: xg[h][r*128+p, q*256+d] = X[r*1024 + h*512 + q*128 + p, d]
    x5 = xbf.reshape(NCORES, 2, HC, P, D)           # [r, h, q, p, d]
    xgs = [np.ascontiguousarray(
        x5[:, h].transpose(0, 2, 1, 3).reshape(NCORES * P, HC * D))
        for h in range(2)]
    wm = np.ascontiguousarray(
        W.reshape(NLAYERS * KPOLY, 2, P, D).reshape(NLAYERS * KPOLY * 2, P, D)
    ).astype(BF16)

    in_maps = []
    for r in range(NCORES):
        rows = slice(r * ROWS, (r + 1) * ROWS)
        lap_blk = (-adj_mat[rows] / degree[rows, None]).astype(BF16)   # [ROWS, N]
        bp = np.ascontiguousarray(lap_blk.T)                           # [N, ROWS]
        xloc = xbf[rows]
        in_maps.append({
            "bp": bp,
            "xg0": xgs[0],
            "xg1": xgs[1],
            "xt": np.ascontiguousarray(xloc.T),
            "w": wm,
            "b": np.ascontiguousarray(b[:, rows, :]),
        })

    res = bass_utils.run_bass_kernel_spmd(
        nc, in_maps, core_ids=list(range(NCORES)),
        trace=bool(int(os.environ.get("CHEB_TRACE", "0"))))
    kernel.last_exec_time_ns = res.exec_time_ns
    out = np.concatenate([res.results[r]["out"] for r in range(NCORES)], axis=0)
    return out


kernel.last_exec_time_ns = None
